# revision 77
# baseline (speedup 1.0000x reference)
"""ChannelAttention Trainium2 kernel (self-contained).

Problem: B=16, H=W=64 (N=4096 tokens), C=512, heads=8, d=64, fp32.
  qkv = x @ qkv_w (+bias);  q,k l2-normalized over tokens;
  attn = softmax((q*exp(scale))^T k);  out = attn @ v^T;  y = out @ proj_w + b.

Sharding: pure data-parallel, 2 batches per core on 8 cores. No collectives.

Algorithm (per batch) — restructured to halve the matmul FLOPs vs the
direct formulation:
  1. G = x^T x                      [C, C]    (contract over N tokens)
  2. T = G @ W_qk                   [C, 2C]   (W_qk = per-head [q|k] columns)
  3. A_h = W_qk_h^T T_h             [128,128] per head = Gram of [q_h|k_h]
     -> diag gives the l2 norms, off-diag block gives q^T k.
  4. W_eff = sum_h W_v_h attn_h^T W_p_h   [C, C]  (head-pair-stacked matmuls)
  5. y^T = W_eff^T x^T              [C, N]   (host transposes back)
Biases (zero in this problem) are handled via gated correction terms.

The two batches are software-pipelined in emission order (the PE executes
in program order): batch b's DVE/ACT softmax section is covered by batch
b+1's G matmuls, and the tiny mid-softmax PE ops (pa2) are spliced into
the middle of the other batch's long matmul phases.

Layouts: x ships twice — token-major bf16 (for G) and channel-major bf16
(for step 5). y returns transposed bf16; host upcasts + transposes back.
"""

import os
import numpy as np

P = 128
C = 512
CCH = C // P            # 4 channel tiles
HEADS = 8
NPAIR = HEADS // 2      # 4 head pairs
D = 64
EPS = 1.55e-5
N_CORES = 8

_CACHE = {}


def _build(nb, n, es, add_acorr, add_bv, add_bp):
    """Build + compile the per-core Bass kernel.

    nb: batches per core; n: tokens per batch; es: tuple of 8 python floats
    (exp(scale), baked); add_*: whether bias corrections are emitted.
    """
    from contextlib import ExitStack
    import concourse.bass as bass  # noqa: F401  (registers engine classes)
    from concourse import bacc
    import concourse.mybir as mybir
    import concourse.tile as tile
    from concourse.masks import make_identity

    f32 = mybir.dt.float32
    f32r = mybir.dt.float32r
    bf16 = mybir.dt.bfloat16
    f8 = mybir.dt.float8e4
    X = mybir.AxisListType.X
    AF = mybir.ActivationFunctionType

    nt = n // P             # token tiles per batch (32)
    ndma = nt // 4          # x DMAs per batch (4 token tiles each)
    nch = n // 512          # 512-token chunks per batch (8)

    nc = bacc.Bacc("TRN2", target_bir_lowering=False)

    x_d = nc.dram_tensor("x", [nb, n, C], f8, kind="ExternalInput")
    xt_d = nc.dram_tensor("xt", [nb, C, n], bf16, kind="ExternalInput")
    wqk_d = nc.dram_tensor("wqk", [P, CCH, 2 * C], f32r, kind="ExternalInput")
    wvt_d = nc.dram_tensor("wvt", [P, NPAIR, C], f32r, kind="ExternalInput")
    wp_d = nc.dram_tensor("wp", [P, NPAIR, C], f32r, kind="ExternalInput")
    yt_d = nc.dram_tensor("yt", [nb, C, n], bf16, kind="ExternalOutput")
    if add_acorr:
        acorr_d = nc.dram_tensor(
            "acorr", [nb, HEADS, P, P], f32, kind="ExternalInput")
    if add_bv:
        bv_d = nc.dram_tensor("bv", [P, NPAIR], f32, kind="ExternalInput")
    if add_bp:
        bp_d = nc.dram_tensor("bp", [P, CCH], f32, kind="ExternalInput")

    with tile.TileContext(nc) as tc, ExitStack() as ctx:
        consts = ctx.enter_context(tc.tile_pool(name="consts", bufs=1))
        x_pool = ctx.enter_context(tc.tile_pool(name="xp", bufs=12))
        xt_pool = ctx.enter_context(tc.tile_pool(name="xtp", bufs=8))
        xtpre_pool = ctx.enter_context(tc.tile_pool(name="xtpre", bufs=4))
        g_pool = ctx.enter_context(tc.tile_pool(name="gp", bufs=2))
        t_pool = ctx.enter_context(tc.tile_pool(name="tp", bufs=1))
        u_pool = ctx.enter_context(tc.tile_pool(name="up", bufs=NPAIR + 1))
        w_pool = ctx.enter_context(tc.tile_pool(name="wp", bufs=2))
        y_pool = ctx.enter_context(tc.tile_pool(name="yp", bufs=4))
        sm_pool = ctx.enter_context(tc.tile_pool(name="smp", bufs=2))
        tin_pool = ctx.enter_context(tc.tile_pool(name="tinp", bufs=NPAIR + 1))
        a_pool = ctx.enter_context(tc.tile_pool(name="ap", bufs=2))
        ac_pool = ctx.enter_context(tc.tile_pool(name="acp", bufs=2))
        pgram = ctx.enter_context(tc.tile_pool(name="pgram", bufs=3, space="PSUM"))
        pmm = ctx.enter_context(tc.tile_pool(name="pmm", bufs=3, space="PSUM"))
        pw = ctx.enter_context(tc.tile_pool(name="pw", bufs=2, space="PSUM"))

        # --- resident constants ---
        # weight DMAs are deferred (emitted mid-G0 / after em_norms(0)) so
        # they don't compete with the x stream at startup
        wqk_sb = consts.tile([P, CCH, 2 * C], f32r)
        wvt_sb = consts.tile([P, NPAIR, C], f32r)
        wp_sb = consts.tile([P, NPAIR, C], f32r)
        ident = consts.tile([P, P], f32)
        zero128 = consts.tile([P, P], f32)
        ioff = consts.tile([P, D], f32)
        es_sb = consts.tile([P, HEADS], f32)
        bv_sb = consts.tile([P, NPAIR], f32) if add_bv else None
        bp_sb = consts.tile([P, CCH], f32) if add_bp else None

        def em_preamble():
            """Init ops for small consts — deferred so the first engine
            instructions are the x-stream DMA dispatches."""
            make_identity(nc, ident[:])
            nc.vector.memset(zero128[:], 0.0)
            # ident_off: rows 64+j have 1 at col j (for diag(s_k))
            nc.gpsimd.memset(ioff[:], 0.0)
            nc.gpsimd.affine_select(
                out=ioff[:], in_=ioff[:],
                compare_op=mybir.AluOpType.not_equal,
                fill=1.0, base=-D, pattern=[[-1, D]], channel_multiplier=1,
            )
            # es_sb[p, h] = exp(scale_h) on the q half, 1.0 on the k half
            nc.gpsimd.memset(es_sb[D:P, :], 1.0)
            for h in range(HEADS):
                nc.gpsimd.memset(es_sb[0:D, h:h + 1], es[h])
            if add_bv:
                nc.gpsimd.dma_start(out=bv_sb[:], in_=bv_d[:])
            if add_bp:
                nc.gpsimd.dma_start(out=bp_sb[:], in_=bp_d[:])

        # per-batch state carried between pipeline stages
        st = [dict() for _ in range(nb)]

        def em_G(b, hooks=None):
            """G = x^T x; hooks[gi]() spliced after DMA group gi.

            First batch's first groups are split small so the PE starts as
            soon as the first 128KB lands.
            """
            x_r = x_d[b].rearrange("(nt p) c -> p nt c", p=P)
            # co=0..2 in the dedicated gram banks; the narrow co=3
            # accumulator borrows a pmm slot (frees a PSUM bank for pmm)
            gps = [pgram.tile([P, C - co * P], f32, tag="g",
                              name=f"g{b}_{co}")
                   for co in range(CCH - 1)]
            gps.append(pmm.tile([P, P], f32, tag="pm", name=f"g{b}_3"))
            # each dma_start uses one HW queue at ~14 GB/s and ~1us dispatch:
            # many small transfers in flight (256KB takes ~19us on a queue,
            # consumed in ~1.7us -> need ~11+ in flight at steady state)
            groups = [1] * 32 if b == 0 else [2] * 16
            startup_engines = [nc.sync, nc.scalar, nc.gpsimd]
            t = 0
            for gi, gsz in enumerate(groups):
                x_t = x_pool.tile([P, 2, C], f8, tag="x")
                if b == 0 and gi < 4:
                    # 3 dispatch engines x half-tile pieces at startup:
                    # fills DMA queues ~6x faster than one engine
                    e1 = startup_engines[gi % 3]
                    e2 = startup_engines[(gi + 1) % 3]
                    e1.dma_start(
                        out=x_t[:, 0:1, 0:256], in_=x_r[:, t:t + 1, 0:256])
                    e2.dma_start(
                        out=x_t[:, 0:1, 256:512],
                        in_=x_r[:, t:t + 1, 256:512])
                elif b == 0:
                    # G0 is DMA-bound: single-tile transfers (low latency),
                    # alternating dispatch engines (2x dispatch rate)
                    deng = nc.sync if gi % 2 == 0 else nc.scalar
                    deng.dma_start(
                        out=x_t[:, 0:1, :], in_=x_r[:, t:t + 1, :])
                else:
                    nc.sync.dma_start(
                        out=x_t[:, 0:gsz, :], in_=x_r[:, t:t + gsz, :])
                for tt in range(gsz):
                    # G is symmetric: only the upper block-triangle is
                    # computed (rhs starts at co*P); lower blocks are
                    # reconstructed by PE transposes below
                    for co in range(CCH):
                        nc.tensor.matmul(
                            gps[co][:],
                            x_t[:, tt, co * P:(co + 1) * P],
                            x_t[:, tt, co * P:],
                            start=(t == 0), stop=(t == nt - 1),
                        )
                    t += 1
                if b == 0 and gi == 6:
                    em_preamble()
                if b == 0 and gi in (7, 8, 9, 10):
                    # wqk fetched as 4 x 512KB pieces on gpsimd so no single
                    # queue is hogged while the x stream ramps
                    q = gi - 7
                    nc.gpsimd.dma_start(
                        wqk_sb[:, q, :], wqk_d[:, q, :])
                if hooks and gi in hooks:
                    hooks[gi]()
            g_sb = g_pool.tile([P, CCH, C], f32r, tag="g")
            for co in range(CCH):
                if co % 2 == 0:
                    nc.vector.tensor_copy(
                        out=g_sb[:, co, co * P:], in_=gps[co][:])
                else:
                    nc.scalar.copy(out=g_sb[:, co, co * P:], in_=gps[co][:])
            for j in range(CCH):
                for i in range(j + 1, CCH):
                    ptr = pmm.tile([P, P], f32, tag="pm")
                    nc.tensor.transpose(
                        ptr[:], g_sb[:, j, i * P:(i + 1) * P].bitcast(f32),
                        ident[:])
                    if (i + j) % 2 == 0:
                        nc.vector.tensor_copy(
                            out=g_sb[:, i, j * P:(j + 1) * P], in_=ptr[:])
                    else:
                        nc.scalar.copy(
                            out=g_sb[:, i, j * P:(j + 1) * P], in_=ptr[:])
            st[b]["g_sb"] = g_sb

        def em_T(b, sub_emits=None):
            """T = G @ W_qk; sub_emits[k]() fires after matmul group k so
            another batch's DVE chain interleaves into the engine queues."""
            g_sb = st[b]["g_sb"]
            t_sb = t_pool.tile([P, CCH, 2 * C], f32r, tag="t")
            k = 0
            for fc in range(2):
                for j in range(CCH):
                    pt = pmm.tile([P, C], f32, tag="pm")
                    for i in range(CCH):
                        nc.tensor.matmul(
                            pt[:],
                            g_sb[:, i, j * P:(j + 1) * P],
                            wqk_sb[:, i, fc * C:(fc + 1) * C],
                            start=(i == 0), stop=(i == CCH - 1),
                        )
                    if k % 2 == 0:
                        nc.vector.tensor_copy(
                            out=t_sb[:, j, fc * C:(fc + 1) * C], in_=pt[:])
                    else:
                        nc.scalar.copy(
                            out=t_sb[:, j, fc * C:(fc + 1) * C], in_=pt[:])
                    if sub_emits and k < len(sub_emits):
                        sub_emits[k]()
                    k += 1
            st[b]["t_sb"] = t_sb

        def em_A_head(b, h):
            """One head's A_h = Gram of [q_h|k_h] (4 matmuls + eviction)."""
            t_sb = st[b]["t_sb"]
            if h == 0:
                st[b]["a_sb"] = a_pool.tile(
                    [P, HEADS, P], f32, tag="A", name=f"A{b}")
            a_sb = st[b]["a_sb"]
            pa = pmm.tile([P, P], f32, tag="pm")
            for i in range(CCH):
                nc.tensor.matmul(
                    pa[:],
                    wqk_sb[:, i, h * P:(h + 1) * P],
                    t_sb[:, i, h * P:(h + 1) * P],
                    start=(i == 0), stop=(i == CCH - 1),
                )
            if add_acorr:
                ac = ac_pool.tile([P, P], f32, tag="ac")
                nc.sync.dma_start(out=ac[:], in_=acorr_d[b, h])
                nc.vector.tensor_add(
                    out=a_sb[:, h, :], in0=pa[:], in1=ac[:])
            elif h % 2 == 0:
                nc.vector.tensor_copy(out=a_sb[:, h, :], in_=pa[:])
            else:
                nc.scalar.copy(out=a_sb[:, h, :], in_=pa[:])

        def em_A(b):
            for h in range(HEADS):
                em_A_head(b, h)

        def em_norms(b):
            """l2 norms for all heads + diag(s_k) prep (DVE/ACT, batched)."""
            a_sb = st[b]["a_sb"]
            s_all = sm_pool.tile([P, HEADS], f32, tag="s")
            for h in range(HEADS):
                dtmp = sm_pool.tile([P, P], f32, tag="dtmp")
                nc.vector.tensor_mul(dtmp[:], a_sb[:, h, :], ident[:])
                nc.vector.reduce_sum(
                    out=s_all[:, h:h + 1], in_=dtmp[:], axis=X)
            nc.vector.tensor_scalar_max(
                out=s_all[:], in0=s_all[:], scalar1=EPS)
            srt = sm_pool.tile([P, HEADS], f32, tag="srt")
            nc.scalar.activation(out=srt[:], in_=s_all[:], func=AF.Sqrt)
            rs = sm_pool.tile([P, HEADS], f32, tag="rs")
            nc.vector.reciprocal(out=rs[:], in_=srt[:])
            # fold exp(scale_h) into the q-side reciprocal norms
            nc.vector.tensor_mul(rs[:], rs[:], es_sb[:])
            dsk = sm_pool.tile([P, HEADS, D], f32, tag="dsk")
            for h in range(HEADS):
                nc.vector.tensor_scalar_mul(
                    out=dsk[D:P, h, :], in0=ioff[D:P, :],
                    scalar1=rs[D:P, h:h + 1])
            st[b]["rs"] = rs
            st[b]["dsk"] = dsk

        def em_pa2(b):
            """attn_pre[dd, e] = (q^T k)[dd, e]*s_k[e]  (tiny PE matmuls)."""
            a_sb, dsk = st[b]["a_sb"], st[b]["dsk"]
            pa2 = pw.tile([D, HEADS * D], f32, tag="pw", name=f"pa2_{b}")
            for h in range(HEADS):
                nc.tensor.matmul(
                    pa2[:, h * D:(h + 1) * D],
                    a_sb[D:P, h, 0:D],
                    dsk[D:P, h, :],
                    start=True, stop=True,
                )
            st[b]["pa2"] = pa2

        def _soft_asb(b, h0, h1):
            rs, pa2 = st[b]["rs"], st[b]["pa2"]
            if h0 == 0:
                st[b]["asb"] = sm_pool.tile(
                    [D, HEADS, D], f32, tag="asb", name=f"asb{b}")
            asb = st[b]["asb"]
            for h in range(h0, h1):
                nc.vector.tensor_scalar_mul(
                    out=asb[:, h, :], in0=pa2[:, h * D:(h + 1) * D],
                    scalar1=rs[0:D, h:h + 1])

        def _soft_nm(b):
            asb = st[b]["asb"]
            nm = sm_pool.tile([D, HEADS], f32, tag="nm", name=f"nm{b}")
            for h in range(HEADS):
                nc.vector.tensor_reduce(
                    out=nm[:, h:h + 1], in_=asb[:, h, :],
                    op=mybir.AluOpType.max, axis=X, negate=True)
            st[b]["nm"] = nm

        def _soft_exp(b):
            asb, nm = st[b]["asb"], st[b]["nm"]
            ex = sm_pool.tile([D, HEADS, D], f32, tag="ex", name=f"ex{b}")
            zsum = sm_pool.tile([D, HEADS], f32, tag="zsum", name=f"zs{b}")
            for h in range(HEADS):
                nc.scalar.activation(
                    out=ex[:, h, :], in_=asb[:, h, :], func=AF.Exp,
                    bias=nm[:, h:h + 1], scale=1.0,
                    accum_out=zsum[:, h:h + 1])
            st[b]["ex"] = ex
            st[b]["zsum"] = zsum

        def _soft_rinv(b):
            rinv = sm_pool.tile([D, HEADS], f32, tag="rinv", name=f"ri{b}")
            nc.vector.reciprocal(out=rinv[:], in_=st[b]["zsum"][:])
            st[b]["rinv"] = rinv

        def _tins_pair(b, g):
            ex, rinv = st[b]["ex"], st[b]["rinv"]
            if g == 0:
                st[b]["tins"] = []
            tin = tin_pool.tile([P, P], f32r, tag="tin", name=f"tin{b}_{g}")
            nc.vector.tensor_copy(out=tin[:], in_=zero128[:])
            for hh in range(2):
                h = 2 * g + hh
                nc.vector.tensor_scalar_mul(
                    out=tin[hh * D:(hh + 1) * D, hh * D:(hh + 1) * D],
                    in0=ex[:, h, :], scalar1=rinv[0:D, h:h + 1])
            st[b]["tins"].append(tin)

        def soft_tins_thunks(b):
            """softmax + tin assembly as 8 thunks for em_T interleaving."""
            return [
                lambda: _soft_asb(b, 0, 4),
                lambda: _soft_asb(b, 4, HEADS),
                lambda: _soft_nm(b),
                lambda: _soft_exp(b),
                lambda: _soft_rinv(b),
                lambda: (_tins_pair(b, 0), _tins_pair(b, 1)),
                lambda: (_tins_pair(b, 2), _tins_pair(b, 3)),
                lambda: None,
            ]

        def em_soft(b):
            _soft_asb(b, 0, 4)
            _soft_asb(b, 4, HEADS)
            _soft_nm(b)
            _soft_exp(b)
            _soft_rinv(b)

        def em_tins(b):
            for g in range(NPAIR):
                _tins_pair(b, g)

        def em_U(b):
            """U_pair = attn_pair^T @ W_p_pair for all pairs."""
            tins = st[b]["tins"]
            u_sbs = []
            for g in range(NPAIR):
                pu = pmm.tile([P, C], f32, tag="pm")
                nc.tensor.matmul(
                    pu[:], tins[g][:], wp_sb[:, g, :], start=True, stop=True)
                u_sb = u_pool.tile([P, C], f32r, tag="u", name=f"u{b}_{g}")
                if g % 2 == 0:
                    nc.vector.tensor_copy(out=u_sb[:], in_=pu[:])
                else:
                    nc.scalar.copy(out=u_sb[:], in_=pu[:])
                u_sbs.append(u_sb)
            st[b]["u_sbs"] = u_sbs

        def em_Weff(b):
            u_sbs = st[b]["u_sbs"]
            weff_sb = w_pool.tile([P, CCH, C], bf16, tag="weff")
            for j in range(CCH):
                pwj = pw.tile([P, C], f32, tag="pw")
                for g in range(NPAIR):
                    nc.tensor.matmul(
                        pwj[:],
                        wvt_sb[:, g, j * P:(j + 1) * P],
                        u_sbs[g][:],
                        start=(g == 0), stop=(g == NPAIR - 1),
                    )
                nc.vector.tensor_copy(out=weff_sb[:, j, :], in_=pwj[:])
            st[b]["weff_sb"] = weff_sb
            # bias row r[f] = sum_e bv[e] U[e, f] (+ proj_b): gated
            if add_bv or add_bp:
                rfull = sm_pool.tile([P, CCH], f32, tag="rf")
                if add_bv:
                    pr = pmm.tile([P, CCH], f32, tag="pm")
                    for jf in range(CCH):
                        for g in range(NPAIR):
                            nc.tensor.matmul(
                                pr[:, jf:jf + 1],
                                u_sbs[g][:, jf * P:(jf + 1) * P],
                                bv_sb[:, g:g + 1],
                                start=(g == 0), stop=(g == NPAIR - 1),
                            )
                    if add_bp:
                        nc.vector.tensor_add(
                            out=rfull[:], in0=pr[:], in1=bp_sb[:])
                    else:
                        nc.vector.tensor_copy(out=rfull[:], in_=pr[:])
                else:
                    nc.vector.tensor_copy(out=rfull[:], in_=bp_sb[:])
                st[b]["rfull"] = rfull

        def em_apply(b, hooks=None):
            """y^T = W_eff^T @ x^T; hooks[ch]() spliced after chunk ch."""
            weff_sb = st[b]["weff_sb"]
            rfull = st[b].get("rfull")
            xt_r = xt_d[b].rearrange("(co p) n -> p co n", p=P)
            yt_r = yt_d[b].rearrange("(ft p) n -> p ft n", p=P)
            pre = st[b].get("xtpre", {})
            for ch in range(nch):
                if ch in pre:
                    xT_t = pre[ch]
                else:
                    xT_t = xt_pool.tile([P, CCH, 512], bf16, tag="xT")
                    nc.sync.dma_start(
                        out=xT_t[:], in_=xt_r[:, :, ch * 512:(ch + 1) * 512])
                y_sb = y_pool.tile([P, CCH, 512], bf16, tag="y")
                for ft in range(CCH):
                    py = pmm.tile([P, 512], f32, tag="pm")
                    for i in range(CCH):
                        nc.tensor.matmul(
                            py[:],
                            weff_sb[:, i, ft * P:(ft + 1) * P],
                            xT_t[:, i, :],
                            start=(i == 0), stop=(i == CCH - 1),
                        )
                    if rfull is not None:
                        nc.vector.tensor_scalar(
                            out=y_sb[:, ft, :], in0=py[:],
                            scalar1=rfull[:, ft:ft + 1], scalar2=None,
                            op0=mybir.AluOpType.add)
                    elif ft % 2 == 0:
                        nc.scalar.copy(out=y_sb[:, ft, :], in_=py[:])
                    else:
                        nc.vector.tensor_copy(out=y_sb[:, ft, :], in_=py[:])
                    # per-ft out DMA: 4 parallel queues, gpsimd dispatch;
                    # final chunks split 2-way for a faster drain
                    if ch >= nch - 2:
                        nc.gpsimd.dma_start(
                            out=yt_r[:, ft:ft + 1, ch * 512:ch * 512 + 256],
                            in_=y_sb[:, ft:ft + 1, 0:256])
                        nc.sync.dma_start(
                            out=yt_r[:, ft:ft + 1, ch * 512 + 256:(ch + 1) * 512],
                            in_=y_sb[:, ft:ft + 1, 256:512])
                    else:
                        nc.gpsimd.dma_start(
                            out=yt_r[:, ft:ft + 1, ch * 512:(ch + 1) * 512],
                            in_=y_sb[:, ft:ft + 1, :])
                if hooks and ch in hooks:
                    hooks[ch]()

        def em_wconsts():
            for q in range(0, NPAIR, 2):
                nc.gpsimd.dma_start(
                    wvt_sb[:, q:q + 2, :], wvt_d[:, q:q + 2, :])
                nc.gpsimd.dma_start(
                    wp_sb[:, q:q + 2, :], wp_d[:, q:q + 2, :])

        def em_xtpre(b, chunks):
            """Prefetch xt chunks of a later batch during the DMA-quiet
            early window (the apply region runs at the DMA ceiling)."""
            xt_r = xt_d[b].rearrange("(co p) n -> p co n", p=P)
            pre = st[b].setdefault("xtpre", {})
            for ch in chunks:
                xT_t = xtpre_pool.tile([P, CCH, 512], bf16, tag="xTp",
                                       name=f"xtp{b}_{ch}")
                nc.gpsimd.dma_start(
                    out=xT_t[:, 0:2, :],
                    in_=xt_r[:, 0:2, ch * 512:(ch + 1) * 512])
                nc.gpsimd.dma_start(
                    out=xT_t[:, 2:4, :],
                    in_=xt_r[:, 2:4, ch * 512:(ch + 1) * 512])
                pre[ch] = xT_t

        # ---- software-pipelined emission over batches ----
        # A-head matmuls of batch b are spread between the long matmul
        # groups of the adjacent batch to keep PE activity high (HAM).
        if nb == 2:
            em_G(0)
            em_T(0)
            em_wconsts()
            g1_hooks = {gi: (lambda hh=gi: em_A_head(0, hh))
                        for gi in range(HEADS)}
            g1_hooks[HEADS] = lambda: em_norms(0)
            g1_hooks[11] = lambda: em_pa2(0)
            em_G(1, hooks=g1_hooks)
            em_xtpre(1, range(4))
            em_T(1, sub_emits=soft_tins_thunks(0))
            em_U(0)
            em_A_head(1, 0)
            em_A_head(1, 1)
            em_Weff(0)
            ap0_hooks = {ch: (lambda cc=ch: (em_A_head(1, 2 * cc + 2),
                                             em_A_head(1, 2 * cc + 3)))
                         for ch in range(3)}
            ap0_hooks[3] = lambda: em_norms(1)
            ap0_hooks[4] = lambda: (em_pa2(1), em_soft(1))
            ap0_hooks[5] = lambda: em_tins(1)
            ap0_hooks[6] = lambda: em_U(1)
            em_apply(0, hooks=ap0_hooks)
            em_Weff(1)
            em_apply(1)
        else:
            for b in range(nb):
                em_G(b, hooks=(
                    {3: (lambda bb=b - 1: em_pa2(bb))} if b > 0 else None))
                if b > 0:
                    em_soft(b - 1)
                    em_tins(b - 1)
                    em_U(b - 1)
                    em_Weff(b - 1)
                em_T(b)
                em_A(b)
                em_norms(b)
                if b == 0:
                    em_wconsts()
                if b > 0:
                    em_apply(b - 1, hooks={1: (lambda bb=b: em_pa2(bb))})
            last = nb - 1
            if nb == 1:
                em_pa2(last)
            em_soft(last)
            em_tins(last)
            em_U(last)
            em_Weff(last)
            em_apply(last)

    nc.compile()
    return nc


def _get_nc(nb, n, es, add_acorr, add_bv, add_bp):
    key = (nb, n, es, add_acorr, add_bv, add_bp)
    if key not in _CACHE:
        _CACHE[key] = _build(nb, n, es, add_acorr, add_bv, add_bp)
    return _CACHE[key]


def prep_inputs(x, qkv_w, q_bias, v_bias, scale, proj_w, proj_b,
                n_cores=N_CORES):
    """Host-side shard + layout prep. Returns (in_maps, es, gates, meta)."""
    import ml_dtypes

    B, H, W, Cc = x.shape
    assert Cc == C
    n = H * W
    nb = B // n_cores

    xf = np.asarray(x, np.float32).reshape(B, n, C)
    # token-major fp8 copy for the G = x^T x pass (errors average out over
    # the 4096-token contraction); channel-major bf16 for y = x @ W_eff
    x8 = xf.astype(ml_dtypes.float8_e4m3)
    xt = np.ascontiguousarray(
        xf.transpose(0, 2, 1)).astype(ml_dtypes.bfloat16)

    w3 = np.asarray(qkv_w, np.float32).reshape(C, HEADS, 3, D)
    wqk = np.ascontiguousarray(w3[:, :, 0:2, :].reshape(C, 2 * C))
    wqk_r = np.ascontiguousarray(
        wqk.reshape(CCH, P, 2 * C).transpose(1, 0, 2))
    wv = w3[:, :, 2, :].reshape(C, C)
    wvt_r = np.ascontiguousarray(
        np.ascontiguousarray(wv.T).reshape(NPAIR, P, C).transpose(1, 0, 2))
    wp_r = np.ascontiguousarray(
        np.asarray(proj_w, np.float32).reshape(NPAIR, P, C).transpose(1, 0, 2))

    # biases exactly as the reference applies them: concat([q_bias, 0, v_bias])
    # indexed by the raw qkv feature id
    bias_full = np.concatenate(
        [q_bias, np.zeros_like(q_bias), v_bias]).astype(np.float32)
    b3 = bias_full.reshape(HEADS, 3, D)
    bqk = b3[:, 0:2, :].reshape(HEADS, P)      # per-head [q|k] bias
    bv = b3[:, 2, :].reshape(C)                # v bias, (h, d) indexed
    bp = np.asarray(proj_b, np.float32).reshape(C)

    add_acorr = bool(np.any(bqk))
    add_bv = bool(np.any(bv))
    add_bp = bool(np.any(bp))
    es = tuple(float(v) for v in
               np.exp(np.asarray(scale, np.float32)).reshape(HEADS))

    acorr = None
    if add_acorr:
        # A_h correction for qkv bias: with z = [q|k] = x W_h + 1 b_h^T,
        # z^T z = W^T G W + b m^T + m b^T + N b b^T,  m = (1^T x) W_h
        sx = xf.sum(axis=1)                    # [B, C]
        wqk_h = wqk.reshape(C, HEADS, P)       # [c, h, f]
        m = np.einsum("bc,chf->bhf", sx, wqk_h)  # [B, HEADS, 128]
        acorr = (m[:, :, None, :] * bqk[None, :, :, None]
                 + m[:, :, :, None] * bqk[None, :, None, :]
                 + float(n) * bqk[None, :, :, None] * bqk[None, :, None, :]
                 ).astype(np.float32)          # [B, HEADS, 128, 128]
    bv_t = np.ascontiguousarray(
        bv.reshape(NPAIR, P).T).astype(np.float32) if add_bv else None
    bp_t = np.ascontiguousarray(
        bp.reshape(CCH, P).T).astype(np.float32) if add_bp else None

    in_maps = []
    for core in range(n_cores):
        sl = slice(core * nb, (core + 1) * nb)
        m_ = {
            "x": np.ascontiguousarray(x8[sl]),
            "xt": np.ascontiguousarray(xt[sl]),
            "wqk": wqk_r, "wvt": wvt_r, "wp": wp_r,
        }
        if add_acorr:
            m_["acorr"] = np.ascontiguousarray(acorr[sl])
        if add_bv:
            m_["bv"] = bv_t
        if add_bp:
            m_["bp"] = bp_t
        in_maps.append(m_)
    return in_maps, es, (add_acorr, add_bv, add_bp), (B, H, W, nb, n)


def kernel(x, qkv_w, q_bias, v_bias, scale, proj_w, proj_b):
    from concourse.bass_utils import run_bass_kernel_spmd

    in_maps, es, gates, (B, H, W, nb, n) = prep_inputs(
        x, qkv_w, q_bias, v_bias, scale, proj_w, proj_b)
    nc = _get_nc(nb, n, es, *gates)
    res = run_bass_kernel_spmd(
        nc, in_maps, core_ids=list(range(N_CORES)),
        trace=bool(int(os.environ.get("KERNEL_TRACE", "0"))),
    )
    yt = np.concatenate([r["yt"] for r in res.results], axis=0)  # [B, C, n]
    out = np.ascontiguousarray(
        yt.astype(np.float32).transpose(0, 2, 1)).reshape(B, H, W, C)
    kernel.last_results = res
    return out


# revision 81
# speedup vs baseline: 1.0451x; 1.0451x over previous
"""ChannelAttention Trainium2 kernel (self-contained).

Problem: B=16, H=W=64 (N=4096 tokens), C=512, heads=8, d=64, fp32.
  qkv = x @ qkv_w (+bias);  q,k l2-normalized over tokens;
  attn = softmax((q*exp(scale))^T k);  out = attn @ v^T;  y = out @ proj_w + b.

Sharding: pure data-parallel, 2 batches per core on 8 cores. No collectives.

Algorithm (per batch) — restructured to halve the matmul FLOPs vs the
direct formulation:
  1. G = x^T x                      [C, C]    (contract over N tokens)
  2. T = G @ W_qk                   [C, 2C]   (W_qk = per-head [q|k] columns)
  3. A_h = W_qk_h^T T_h             [128,128] per head = Gram of [q_h|k_h]
     -> diag gives the l2 norms, off-diag block gives q^T k.
  4. W_eff = sum_h W_v_h attn_h^T W_p_h   [C, C]  (head-pair-stacked matmuls)
  5. y^T = W_eff^T x^T              [C, N]   (host transposes back)
Biases (zero in this problem) are handled via gated correction terms.

The two batches are software-pipelined in emission order (the PE executes
in program order): batch b's DVE/ACT softmax section is covered by batch
b+1's G matmuls, and the tiny mid-softmax PE ops (pa2) are spliced into
the middle of the other batch's long matmul phases.

Layouts: x ships twice — token-major bf16 (for G) and channel-major bf16
(for step 5). y returns transposed bf16; host upcasts + transposes back.
"""

import os
import numpy as np

P = 128
C = 512
CCH = C // P            # 4 channel tiles
HEADS = 8
NPAIR = HEADS // 2      # 4 head pairs
D = 64
EPS = 1.55e-5
N_CORES = 8

_CACHE = {}


def _build(nb, n, es, add_acorr, add_bv, add_bp):
    """Build + compile the per-core Bass kernel.

    nb: batches per core; n: tokens per batch; es: tuple of 8 python floats
    (exp(scale), baked); add_*: whether bias corrections are emitted.
    """
    from contextlib import ExitStack
    import concourse.bass as bass  # noqa: F401  (registers engine classes)
    from concourse import bacc
    import concourse.mybir as mybir
    import concourse.tile as tile
    from concourse.masks import make_identity

    f32 = mybir.dt.float32
    f32r = mybir.dt.float32r
    bf16 = mybir.dt.bfloat16
    f8 = mybir.dt.float8e4
    X = mybir.AxisListType.X
    AF = mybir.ActivationFunctionType

    nt = n // P             # token tiles per batch (32)
    ndma = nt // 4          # x DMAs per batch (4 token tiles each)
    nch = n // 512          # 512-token chunks per batch (8)

    nc = bacc.Bacc("TRN2", target_bir_lowering=False)

    x_d = nc.dram_tensor("x", [nb, n, C], f8, kind="ExternalInput")
    xt_d = nc.dram_tensor("xt", [nb, C, n], bf16, kind="ExternalInput")
    wqk_d = nc.dram_tensor("wqk", [P, CCH, 2 * C], f32r, kind="ExternalInput")
    wvt_d = nc.dram_tensor("wvt", [P, NPAIR, C], f32r, kind="ExternalInput")
    wp_d = nc.dram_tensor("wp", [P, NPAIR, C], f32r, kind="ExternalInput")
    yt_d = nc.dram_tensor("yt", [nb, C, n], bf16, kind="ExternalOutput")
    if add_acorr:
        acorr_d = nc.dram_tensor(
            "acorr", [nb, HEADS, P, P], f32, kind="ExternalInput")
    if add_bv:
        bv_d = nc.dram_tensor("bv", [P, NPAIR], f32, kind="ExternalInput")
    if add_bp:
        bp_d = nc.dram_tensor("bp", [P, CCH], f32, kind="ExternalInput")

    with tile.TileContext(nc) as tc, ExitStack() as ctx:
        consts = ctx.enter_context(tc.tile_pool(name="consts", bufs=1))
        x_pool = ctx.enter_context(tc.tile_pool(name="xp", bufs=22))
        xt_pool = ctx.enter_context(tc.tile_pool(name="xtp", bufs=10))
        g_pool = ctx.enter_context(tc.tile_pool(name="gp", bufs=2))
        t_pool = ctx.enter_context(tc.tile_pool(name="tp", bufs=1))
        u_pool = ctx.enter_context(tc.tile_pool(name="up", bufs=NPAIR + 1))
        w_pool = ctx.enter_context(tc.tile_pool(name="wp", bufs=2))
        y_pool = ctx.enter_context(tc.tile_pool(name="yp", bufs=5))
        sm_pool = ctx.enter_context(tc.tile_pool(name="smp", bufs=2))
        tin_pool = ctx.enter_context(tc.tile_pool(name="tinp", bufs=NPAIR + 1))
        a_pool = ctx.enter_context(tc.tile_pool(name="ap", bufs=2))
        ac_pool = ctx.enter_context(tc.tile_pool(name="acp", bufs=2))
        pgram = ctx.enter_context(tc.tile_pool(name="pgram", bufs=3, space="PSUM"))
        pmm = ctx.enter_context(tc.tile_pool(name="pmm", bufs=3, space="PSUM"))
        pw = ctx.enter_context(tc.tile_pool(name="pw", bufs=2, space="PSUM"))

        # --- resident constants ---
        # weight DMAs are deferred (emitted mid-G0 / after em_norms(0)) so
        # they don't compete with the x stream at startup
        wqk_sb = consts.tile([P, CCH, 2 * C], f32r)
        wvt_sb = consts.tile([P, NPAIR, C], f32r)
        wp_sb = consts.tile([P, NPAIR, C], f32r)
        ident = consts.tile([P, P], f32)
        zero128 = consts.tile([P, P], f32)
        ioff = consts.tile([P, D], f32)
        es_sb = consts.tile([P, HEADS], f32)
        bv_sb = consts.tile([P, NPAIR], f32) if add_bv else None
        bp_sb = consts.tile([P, CCH], f32) if add_bp else None

        def em_preamble():
            """Init ops for small consts — deferred so the first engine
            instructions are the x-stream DMA dispatches."""
            make_identity(nc, ident[:])
            nc.vector.memset(zero128[:], 0.0)
            # ident_off: rows 64+j have 1 at col j (for diag(s_k))
            nc.gpsimd.memset(ioff[:], 0.0)
            nc.gpsimd.affine_select(
                out=ioff[:], in_=ioff[:],
                compare_op=mybir.AluOpType.not_equal,
                fill=1.0, base=-D, pattern=[[-1, D]], channel_multiplier=1,
            )
            # es_sb[p, h] = exp(scale_h) on the q half, 1.0 on the k half
            nc.gpsimd.memset(es_sb[D:P, :], 1.0)
            for h in range(HEADS):
                nc.gpsimd.memset(es_sb[0:D, h:h + 1], es[h])
            if add_bv:
                nc.gpsimd.dma_start(out=bv_sb[:], in_=bv_d[:])
            if add_bp:
                nc.gpsimd.dma_start(out=bp_sb[:], in_=bp_d[:])

        # per-batch state carried between pipeline stages
        st = [dict() for _ in range(nb)]

        def em_G(b, hooks=None):
            """G = x^T x; hooks[gi]() spliced after DMA group gi.

            First batch's first groups are split small so the PE starts as
            soon as the first 128KB lands.
            """
            x_r = x_d[b].rearrange("(nt p) c -> p nt c", p=P)
            # co=0..2 in the dedicated gram banks; the narrow co=3
            # accumulator borrows a pmm slot (frees a PSUM bank for pmm)
            gps = [pgram.tile([P, C - co * P], f32, tag="g",
                              name=f"g{b}_{co}")
                   for co in range(CCH - 1)]
            gps.append(pmm.tile([P, P], f32, tag="pm", name=f"g{b}_3"))
            # each dma_start uses one HW queue at ~14 GB/s and ~1us dispatch:
            # many small transfers in flight (256KB takes ~19us on a queue,
            # consumed in ~1.7us -> need ~11+ in flight at steady state)
            groups = [1] * 32 if b == 0 else [2] * 16
            startup_engines = [nc.sync, nc.scalar, nc.gpsimd]
            t = 0
            for gi, gsz in enumerate(groups):
                x_t = x_pool.tile([P, 2, C], f8, tag="x")
                if b == 0 and gi < 4:
                    # 3 dispatch engines x half-tile pieces at startup:
                    # fills DMA queues ~6x faster than one engine
                    e1 = startup_engines[gi % 3]
                    e2 = startup_engines[(gi + 1) % 3]
                    e1.dma_start(
                        out=x_t[:, 0:1, 0:256], in_=x_r[:, t:t + 1, 0:256])
                    e2.dma_start(
                        out=x_t[:, 0:1, 256:512],
                        in_=x_r[:, t:t + 1, 256:512])
                elif b == 0:
                    # G0 is DMA-bound: single-tile transfers (low latency),
                    # alternating dispatch engines (2x dispatch rate)
                    deng = nc.sync if gi % 2 == 0 else nc.scalar
                    deng.dma_start(
                        out=x_t[:, 0:1, :], in_=x_r[:, t:t + 1, :])
                else:
                    nc.sync.dma_start(
                        out=x_t[:, 0:gsz, :], in_=x_r[:, t:t + gsz, :])
                for tt in range(gsz):
                    # G is symmetric: only the upper block-triangle is
                    # computed (rhs starts at co*P); lower blocks are
                    # reconstructed by PE transposes below
                    for co in range(CCH):
                        nc.tensor.matmul(
                            gps[co][:],
                            x_t[:, tt, co * P:(co + 1) * P],
                            x_t[:, tt, co * P:],
                            start=(t == 0), stop=(t == nt - 1),
                        )
                    t += 1
                if b == 0 and gi == 6:
                    em_preamble()
                if b == 0 and gi in (7, 8, 9, 10):
                    # wqk fetched as 4 x 512KB pieces on gpsimd so no single
                    # queue is hogged while the x stream ramps
                    q = gi - 7
                    nc.gpsimd.dma_start(
                        wqk_sb[:, q, :], wqk_d[:, q, :])
                if hooks and gi in hooks:
                    hooks[gi]()
            g_sb = g_pool.tile([P, CCH, C], f32r, tag="g")
            for co in range(CCH):
                if co % 2 == 0:
                    nc.vector.tensor_copy(
                        out=g_sb[:, co, co * P:], in_=gps[co][:])
                else:
                    nc.scalar.copy(out=g_sb[:, co, co * P:], in_=gps[co][:])
            for j in range(CCH):
                for i in range(j + 1, CCH):
                    ptr = pmm.tile([P, P], f32, tag="pm")
                    nc.tensor.transpose(
                        ptr[:], g_sb[:, j, i * P:(i + 1) * P].bitcast(f32),
                        ident[:])
                    if (i + j) % 2 == 0:
                        nc.vector.tensor_copy(
                            out=g_sb[:, i, j * P:(j + 1) * P], in_=ptr[:])
                    else:
                        nc.scalar.copy(
                            out=g_sb[:, i, j * P:(j + 1) * P], in_=ptr[:])
            st[b]["g_sb"] = g_sb

        def em_T(b, sub_emits=None):
            """T = G @ W_qk; sub_emits[k]() fires after matmul group k so
            another batch's DVE chain interleaves into the engine queues."""
            g_sb = st[b]["g_sb"]
            t_sb = t_pool.tile([P, CCH, 2 * C], f32r, tag="t")
            k = 0
            for fc in range(2):
                for j in range(CCH):
                    pt = pmm.tile([P, C], f32, tag="pm")
                    for i in range(CCH):
                        nc.tensor.matmul(
                            pt[:],
                            g_sb[:, i, j * P:(j + 1) * P],
                            wqk_sb[:, i, fc * C:(fc + 1) * C],
                            start=(i == 0), stop=(i == CCH - 1),
                        )
                    if k % 2 == 0:
                        nc.vector.tensor_copy(
                            out=t_sb[:, j, fc * C:(fc + 1) * C], in_=pt[:])
                    else:
                        nc.scalar.copy(
                            out=t_sb[:, j, fc * C:(fc + 1) * C], in_=pt[:])
                    if sub_emits and k < len(sub_emits):
                        sub_emits[k]()
                    k += 1
            st[b]["t_sb"] = t_sb

        def em_A_head(b, h):
            """One head's A_h = Gram of [q_h|k_h] (4 matmuls + eviction)."""
            t_sb = st[b]["t_sb"]
            if h == 0:
                st[b]["a_sb"] = a_pool.tile(
                    [P, HEADS, P], f32, tag="A", name=f"A{b}")
            a_sb = st[b]["a_sb"]
            pa = pmm.tile([P, P], f32, tag="pm")
            for i in range(CCH):
                nc.tensor.matmul(
                    pa[:],
                    wqk_sb[:, i, h * P:(h + 1) * P],
                    t_sb[:, i, h * P:(h + 1) * P],
                    start=(i == 0), stop=(i == CCH - 1),
                )
            if add_acorr:
                ac = ac_pool.tile([P, P], f32, tag="ac")
                nc.sync.dma_start(out=ac[:], in_=acorr_d[b, h])
                nc.vector.tensor_add(
                    out=a_sb[:, h, :], in0=pa[:], in1=ac[:])
            elif h % 2 == 0:
                nc.vector.tensor_copy(out=a_sb[:, h, :], in_=pa[:])
            else:
                nc.scalar.copy(out=a_sb[:, h, :], in_=pa[:])

        def em_A(b):
            for h in range(HEADS):
                em_A_head(b, h)

        def em_norms(b):
            """l2 norms for all heads + diag(s_k) prep (DVE/ACT, batched)."""
            a_sb = st[b]["a_sb"]
            s_all = sm_pool.tile([P, HEADS], f32, tag="s")
            for h in range(HEADS):
                dtmp = sm_pool.tile([P, P], f32, tag="dtmp")
                nc.vector.tensor_mul(dtmp[:], a_sb[:, h, :], ident[:])
                nc.vector.reduce_sum(
                    out=s_all[:, h:h + 1], in_=dtmp[:], axis=X)
            nc.vector.tensor_scalar_max(
                out=s_all[:], in0=s_all[:], scalar1=EPS)
            srt = sm_pool.tile([P, HEADS], f32, tag="srt")
            nc.scalar.activation(out=srt[:], in_=s_all[:], func=AF.Sqrt)
            rs = sm_pool.tile([P, HEADS], f32, tag="rs")
            nc.vector.reciprocal(out=rs[:], in_=srt[:])
            # fold exp(scale_h) into the q-side reciprocal norms
            nc.vector.tensor_mul(rs[:], rs[:], es_sb[:])
            dsk = sm_pool.tile([P, HEADS, D], f32, tag="dsk")
            for h in range(HEADS):
                nc.vector.tensor_scalar_mul(
                    out=dsk[D:P, h, :], in0=ioff[D:P, :],
                    scalar1=rs[D:P, h:h + 1])
            st[b]["rs"] = rs
            st[b]["dsk"] = dsk

        def em_pa2(b):
            """attn_pre[dd, e] = (q^T k)[dd, e]*s_k[e]  (tiny PE matmuls)."""
            a_sb, dsk = st[b]["a_sb"], st[b]["dsk"]
            pa2 = pw.tile([D, HEADS * D], f32, tag="pw", name=f"pa2_{b}")
            for h in range(HEADS):
                nc.tensor.matmul(
                    pa2[:, h * D:(h + 1) * D],
                    a_sb[D:P, h, 0:D],
                    dsk[D:P, h, :],
                    start=True, stop=True,
                )
            st[b]["pa2"] = pa2

        def _soft_asb(b, h0, h1):
            rs, pa2 = st[b]["rs"], st[b]["pa2"]
            if h0 == 0:
                st[b]["asb"] = sm_pool.tile(
                    [D, HEADS, D], f32, tag="asb", name=f"asb{b}")
            asb = st[b]["asb"]
            for h in range(h0, h1):
                nc.vector.tensor_scalar_mul(
                    out=asb[:, h, :], in0=pa2[:, h * D:(h + 1) * D],
                    scalar1=rs[0:D, h:h + 1])

        def _soft_nm(b):
            asb = st[b]["asb"]
            nm = sm_pool.tile([D, HEADS], f32, tag="nm", name=f"nm{b}")
            for h in range(HEADS):
                nc.vector.tensor_reduce(
                    out=nm[:, h:h + 1], in_=asb[:, h, :],
                    op=mybir.AluOpType.max, axis=X, negate=True)
            st[b]["nm"] = nm

        def _soft_exp(b):
            asb, nm = st[b]["asb"], st[b]["nm"]
            ex = sm_pool.tile([D, HEADS, D], f32, tag="ex", name=f"ex{b}")
            zsum = sm_pool.tile([D, HEADS], f32, tag="zsum", name=f"zs{b}")
            for h in range(HEADS):
                nc.scalar.activation(
                    out=ex[:, h, :], in_=asb[:, h, :], func=AF.Exp,
                    bias=nm[:, h:h + 1], scale=1.0,
                    accum_out=zsum[:, h:h + 1])
            st[b]["ex"] = ex
            st[b]["zsum"] = zsum

        def _soft_rinv(b):
            rinv = sm_pool.tile([D, HEADS], f32, tag="rinv", name=f"ri{b}")
            nc.vector.reciprocal(out=rinv[:], in_=st[b]["zsum"][:])
            st[b]["rinv"] = rinv

        def _tins_pair(b, g):
            ex, rinv = st[b]["ex"], st[b]["rinv"]
            if g == 0:
                st[b]["tins"] = []
            tin = tin_pool.tile([P, P], f32r, tag="tin", name=f"tin{b}_{g}")
            nc.vector.tensor_copy(out=tin[:], in_=zero128[:])
            for hh in range(2):
                h = 2 * g + hh
                nc.vector.tensor_scalar_mul(
                    out=tin[hh * D:(hh + 1) * D, hh * D:(hh + 1) * D],
                    in0=ex[:, h, :], scalar1=rinv[0:D, h:h + 1])
            st[b]["tins"].append(tin)

        def soft_tins_thunks(b):
            """softmax + tin assembly as 8 thunks for em_T interleaving."""
            return [
                lambda: _soft_asb(b, 0, 4),
                lambda: _soft_asb(b, 4, HEADS),
                lambda: _soft_nm(b),
                lambda: _soft_exp(b),
                lambda: _soft_rinv(b),
                lambda: (_tins_pair(b, 0), _tins_pair(b, 1)),
                lambda: (_tins_pair(b, 2), _tins_pair(b, 3)),
                lambda: None,
            ]

        def em_soft(b):
            _soft_asb(b, 0, 4)
            _soft_asb(b, 4, HEADS)
            _soft_nm(b)
            _soft_exp(b)
            _soft_rinv(b)

        def em_tins(b):
            for g in range(NPAIR):
                _tins_pair(b, g)

        def em_U(b):
            """U_pair = attn_pair^T @ W_p_pair for all pairs."""
            tins = st[b]["tins"]
            u_sbs = []
            for g in range(NPAIR):
                pu = pmm.tile([P, C], f32, tag="pm")
                nc.tensor.matmul(
                    pu[:], tins[g][:], wp_sb[:, g, :], start=True, stop=True)
                u_sb = u_pool.tile([P, C], f32r, tag="u", name=f"u{b}_{g}")
                if g % 2 == 0:
                    nc.vector.tensor_copy(out=u_sb[:], in_=pu[:])
                else:
                    nc.scalar.copy(out=u_sb[:], in_=pu[:])
                u_sbs.append(u_sb)
            st[b]["u_sbs"] = u_sbs

        def em_Weff(b):
            u_sbs = st[b]["u_sbs"]
            weff_sb = w_pool.tile([P, CCH, C], bf16, tag="weff")
            for j in range(CCH):
                pwj = pw.tile([P, C], f32, tag="pw")
                for g in range(NPAIR):
                    nc.tensor.matmul(
                        pwj[:],
                        wvt_sb[:, g, j * P:(j + 1) * P],
                        u_sbs[g][:],
                        start=(g == 0), stop=(g == NPAIR - 1),
                    )
                nc.vector.tensor_copy(out=weff_sb[:, j, :], in_=pwj[:])
            st[b]["weff_sb"] = weff_sb
            # bias row r[f] = sum_e bv[e] U[e, f] (+ proj_b): gated
            if add_bv or add_bp:
                rfull = sm_pool.tile([P, CCH], f32, tag="rf")
                if add_bv:
                    pr = pmm.tile([P, CCH], f32, tag="pm")
                    for jf in range(CCH):
                        for g in range(NPAIR):
                            nc.tensor.matmul(
                                pr[:, jf:jf + 1],
                                u_sbs[g][:, jf * P:(jf + 1) * P],
                                bv_sb[:, g:g + 1],
                                start=(g == 0), stop=(g == NPAIR - 1),
                            )
                    if add_bp:
                        nc.vector.tensor_add(
                            out=rfull[:], in0=pr[:], in1=bp_sb[:])
                    else:
                        nc.vector.tensor_copy(out=rfull[:], in_=pr[:])
                else:
                    nc.vector.tensor_copy(out=rfull[:], in_=bp_sb[:])
                st[b]["rfull"] = rfull

        def em_apply(b, hooks=None):
            """y^T = W_eff^T @ x^T; hooks[ch]() spliced after chunk ch."""
            weff_sb = st[b]["weff_sb"]
            rfull = st[b].get("rfull")
            xt_r = xt_d[b].rearrange("(co p) n -> p co n", p=P)
            yt_r = yt_d[b].rearrange("(ft p) n -> p ft n", p=P)
            pre = st[b].get("xtpre", {})
            for ch in range(nch):
                if ch in pre:
                    xT_t = pre[ch]
                else:
                    xT_t = xt_pool.tile([P, CCH, 512], bf16, tag="xT")
                    nc.sync.dma_start(
                        out=xT_t[:], in_=xt_r[:, :, ch * 512:(ch + 1) * 512])
                y_sb = y_pool.tile([P, CCH, 512], bf16, tag="y")
                for ft in range(CCH):
                    py = pmm.tile([P, 512], f32, tag="pm")
                    for i in range(CCH):
                        nc.tensor.matmul(
                            py[:],
                            weff_sb[:, i, ft * P:(ft + 1) * P],
                            xT_t[:, i, :],
                            start=(i == 0), stop=(i == CCH - 1),
                        )
                    if rfull is not None:
                        nc.vector.tensor_scalar(
                            out=y_sb[:, ft, :], in0=py[:],
                            scalar1=rfull[:, ft:ft + 1], scalar2=None,
                            op0=mybir.AluOpType.add)
                    elif ft % 2 == 0:
                        nc.scalar.copy(out=y_sb[:, ft, :], in_=py[:])
                    else:
                        nc.vector.tensor_copy(out=y_sb[:, ft, :], in_=py[:])
                    # per-ft out DMA: 4 parallel queues, gpsimd dispatch;
                    # final chunks split finer for a faster drain
                    if ch == nch - 1:
                        for q in range(4):
                            eng = nc.gpsimd if q % 2 == 0 else nc.sync
                            lo = ch * 512 + q * 128
                            eng.dma_start(
                                out=yt_r[:, ft:ft + 1, lo:lo + 128],
                                in_=y_sb[:, ft:ft + 1,
                                         q * 128:(q + 1) * 128])
                    elif ch == nch - 2:
                        nc.gpsimd.dma_start(
                            out=yt_r[:, ft:ft + 1, ch * 512:ch * 512 + 256],
                            in_=y_sb[:, ft:ft + 1, 0:256])
                        nc.sync.dma_start(
                            out=yt_r[:, ft:ft + 1, ch * 512 + 256:(ch + 1) * 512],
                            in_=y_sb[:, ft:ft + 1, 256:512])
                    else:
                        nc.gpsimd.dma_start(
                            out=yt_r[:, ft:ft + 1, ch * 512:(ch + 1) * 512],
                            in_=y_sb[:, ft:ft + 1, :])
                if hooks and ch in hooks:
                    hooks[ch]()

        def em_wconsts():
            for q in range(0, NPAIR, 2):
                nc.gpsimd.dma_start(
                    wvt_sb[:, q:q + 2, :], wvt_d[:, q:q + 2, :])
                nc.gpsimd.dma_start(
                    wp_sb[:, q:q + 2, :], wp_d[:, q:q + 2, :])

        def em_xtpre(b, chunks):
            """Prefetch xt chunks of a later batch during the DMA-quiet
            early window (the apply region runs at the DMA ceiling)."""
            xt_r = xt_d[b].rearrange("(co p) n -> p co n", p=P)
            pre = st[b].setdefault("xtpre", {})
            for ch in chunks:
                xT_t = xtpre_pool.tile([P, CCH, 512], bf16, tag="xTp",
                                       name=f"xtp{b}_{ch}")
                nc.gpsimd.dma_start(
                    out=xT_t[:, 0:2, :],
                    in_=xt_r[:, 0:2, ch * 512:(ch + 1) * 512])
                nc.gpsimd.dma_start(
                    out=xT_t[:, 2:4, :],
                    in_=xt_r[:, 2:4, ch * 512:(ch + 1) * 512])
                pre[ch] = xT_t

        # ---- software-pipelined emission over batches ----
        # A-head matmuls of batch b are spread between the long matmul
        # groups of the adjacent batch to keep PE activity high (HAM).
        if nb == 2:
            em_G(0)
            em_T(0)
            em_wconsts()
            g1_hooks = {gi: (lambda hh=gi: em_A_head(0, hh))
                        for gi in range(HEADS)}
            g1_hooks[HEADS] = lambda: em_norms(0)
            g1_hooks[11] = lambda: em_pa2(0)
            em_G(1, hooks=g1_hooks)
            em_T(1, sub_emits=soft_tins_thunks(0))
            em_U(0)
            em_A_head(1, 0)
            em_A_head(1, 1)
            em_Weff(0)
            ap0_hooks = {ch: (lambda cc=ch: (em_A_head(1, 2 * cc + 2),
                                             em_A_head(1, 2 * cc + 3)))
                         for ch in range(3)}
            ap0_hooks[3] = lambda: em_norms(1)
            ap0_hooks[4] = lambda: (em_pa2(1), em_soft(1))
            ap0_hooks[5] = lambda: em_tins(1)
            ap0_hooks[6] = lambda: em_U(1)
            em_apply(0, hooks=ap0_hooks)
            em_Weff(1)
            em_apply(1)
        else:
            for b in range(nb):
                em_G(b, hooks=(
                    {3: (lambda bb=b - 1: em_pa2(bb))} if b > 0 else None))
                if b > 0:
                    em_soft(b - 1)
                    em_tins(b - 1)
                    em_U(b - 1)
                    em_Weff(b - 1)
                em_T(b)
                em_A(b)
                em_norms(b)
                if b == 0:
                    em_wconsts()
                if b > 0:
                    em_apply(b - 1, hooks={1: (lambda bb=b: em_pa2(bb))})
            last = nb - 1
            if nb == 1:
                em_pa2(last)
            em_soft(last)
            em_tins(last)
            em_U(last)
            em_Weff(last)
            em_apply(last)

    nc.compile()
    return nc


def _get_nc(nb, n, es, add_acorr, add_bv, add_bp):
    key = (nb, n, es, add_acorr, add_bv, add_bp)
    if key not in _CACHE:
        _CACHE[key] = _build(nb, n, es, add_acorr, add_bv, add_bp)
    return _CACHE[key]


def prep_inputs(x, qkv_w, q_bias, v_bias, scale, proj_w, proj_b,
                n_cores=N_CORES):
    """Host-side shard + layout prep. Returns (in_maps, es, gates, meta)."""
    import ml_dtypes

    B, H, W, Cc = x.shape
    assert Cc == C
    n = H * W
    nb = B // n_cores

    xf = np.asarray(x, np.float32).reshape(B, n, C)
    # token-major fp8 copy for the G = x^T x pass (errors average out over
    # the 4096-token contraction); channel-major bf16 for y = x @ W_eff
    x8 = xf.astype(ml_dtypes.float8_e4m3)
    xt = np.ascontiguousarray(
        xf.transpose(0, 2, 1)).astype(ml_dtypes.bfloat16)

    w3 = np.asarray(qkv_w, np.float32).reshape(C, HEADS, 3, D)
    wqk = np.ascontiguousarray(w3[:, :, 0:2, :].reshape(C, 2 * C))
    wqk_r = np.ascontiguousarray(
        wqk.reshape(CCH, P, 2 * C).transpose(1, 0, 2))
    wv = w3[:, :, 2, :].reshape(C, C)
    wvt_r = np.ascontiguousarray(
        np.ascontiguousarray(wv.T).reshape(NPAIR, P, C).transpose(1, 0, 2))
    wp_r = np.ascontiguousarray(
        np.asarray(proj_w, np.float32).reshape(NPAIR, P, C).transpose(1, 0, 2))

    # biases exactly as the reference applies them: concat([q_bias, 0, v_bias])
    # indexed by the raw qkv feature id
    bias_full = np.concatenate(
        [q_bias, np.zeros_like(q_bias), v_bias]).astype(np.float32)
    b3 = bias_full.reshape(HEADS, 3, D)
    bqk = b3[:, 0:2, :].reshape(HEADS, P)      # per-head [q|k] bias
    bv = b3[:, 2, :].reshape(C)                # v bias, (h, d) indexed
    bp = np.asarray(proj_b, np.float32).reshape(C)

    add_acorr = bool(np.any(bqk))
    add_bv = bool(np.any(bv))
    add_bp = bool(np.any(bp))
    es = tuple(float(v) for v in
               np.exp(np.asarray(scale, np.float32)).reshape(HEADS))

    acorr = None
    if add_acorr:
        # A_h correction for qkv bias: with z = [q|k] = x W_h + 1 b_h^T,
        # z^T z = W^T G W + b m^T + m b^T + N b b^T,  m = (1^T x) W_h
        sx = xf.sum(axis=1)                    # [B, C]
        wqk_h = wqk.reshape(C, HEADS, P)       # [c, h, f]
        m = np.einsum("bc,chf->bhf", sx, wqk_h)  # [B, HEADS, 128]
        acorr = (m[:, :, None, :] * bqk[None, :, :, None]
                 + m[:, :, :, None] * bqk[None, :, None, :]
                 + float(n) * bqk[None, :, :, None] * bqk[None, :, None, :]
                 ).astype(np.float32)          # [B, HEADS, 128, 128]
    bv_t = np.ascontiguousarray(
        bv.reshape(NPAIR, P).T).astype(np.float32) if add_bv else None
    bp_t = np.ascontiguousarray(
        bp.reshape(CCH, P).T).astype(np.float32) if add_bp else None

    in_maps = []
    for core in range(n_cores):
        sl = slice(core * nb, (core + 1) * nb)
        m_ = {
            "x": np.ascontiguousarray(x8[sl]),
            "xt": np.ascontiguousarray(xt[sl]),
            "wqk": wqk_r, "wvt": wvt_r, "wp": wp_r,
        }
        if add_acorr:
            m_["acorr"] = np.ascontiguousarray(acorr[sl])
        if add_bv:
            m_["bv"] = bv_t
        if add_bp:
            m_["bp"] = bp_t
        in_maps.append(m_)
    return in_maps, es, (add_acorr, add_bv, add_bp), (B, H, W, nb, n)


def kernel(x, qkv_w, q_bias, v_bias, scale, proj_w, proj_b):
    from concourse.bass_utils import run_bass_kernel_spmd

    in_maps, es, gates, (B, H, W, nb, n) = prep_inputs(
        x, qkv_w, q_bias, v_bias, scale, proj_w, proj_b)
    nc = _get_nc(nb, n, es, *gates)
    res = run_bass_kernel_spmd(
        nc, in_maps, core_ids=list(range(N_CORES)),
        trace=bool(int(os.environ.get("KERNEL_TRACE", "0"))),
    )
    yt = np.concatenate([r["yt"] for r in res.results], axis=0)  # [B, C, n]
    out = np.ascontiguousarray(
        yt.astype(np.float32).transpose(0, 2, 1)).reshape(B, H, W, C)
    kernel.last_results = res
    return out
